# revision 9
# baseline (speedup 1.0000x reference)
"""Trainium2 kernel for nn_Decoder_70781061038948.

Pipeline: poly_roots (companion-matrix eigvals) -> KAN layer (20->1024) ->
KAN layer (1024->1024).

Device strategy (8 NeuronCores, pure data-parallel over the 8192 batch):
  * Eigenvalues on host CPU with the exact jax/LAPACK call the reference
    uses (LAPACK's eigenvalue ORDER is not reproducible on device, and the
    KAN layers are order-sensitive). ~0% of FLOPs and bytes.
  * Everything else (B-spline features + both KAN layers) runs on the 8
    cores, batch-sharded 1024/core.

Math: uniform-knot cubic B-splines rewritten through the cumulative basis
  6*Qt_j(x) = phi_j - 3 phi_{j+1} + 3 phi_{j+2} - phi_{j+3},
  phi_r = min(relu(2.5x + 5.5 - r), 11 - r)^3
so each KAN layer is ONE dense matmul over 10 features per input
(silu + 9 bounded Qt). Weight differencing and the 1/6 are folded into the
weights on host in float64.

v2 (from NTFF profile of v1):
  * v1 span 443us/iter: DVE 89% busy (slow 2-stream tensor_tensor +
    gpsimd subs), PE 78%, weight DMA 2x42MB/iter at the ~310GB/s DMA cap.
  * Layer-1 weights+features now bf16 (PE rate unchanged, DMA and SBUF
    halved; adds ~2e-3 rel err vs 2e-2 budget).  Layer-0 stays f32r so h
    (the layer-1 spline argument) keeps full precision.
  * Feature combines are scalar_tensor_tensor only (measured 2.2x faster
    than tensor_tensor on DVE), split across DVE and Pool:
       u_g = 3 phi_{j+2} - phi_{j+3};  v_g = phi_j - 3 phi_{j+1}
       qt_g = u_g + v_g   (bf16 out)
  * Deeper weight prefetch (w1p bufs), w0 DMA split into 32-row chunks.
"""

import numpy as np
from contextlib import ExitStack

# ---------------------------------------------------------------- constants
K = 10
B = 8192
CORES = 8
BC = B // CORES            # 1024 batch rows per core
IN0 = 2 * K                # 20
HID = 1024
OUT = 1024
NJ = 9                     # Qt features per input
NSEC = 1 + NJ              # silu + 9 Qt
NR = 12                    # relu-cube shifts r = 0..11
C0_ROWS = 32 * NSEC        # 320 (layer-0 c-layout: 10 sections of 32 rows)
C1_TILES = 8 * NSEC        # 80 (layer-1 c-tiles of 128)

_f32 = np.float32

try:
    import ml_dtypes
    _BF16 = ml_dtypes.bfloat16
except ImportError:  # pragma: no cover
    _BF16 = None


def _round_f32r(a):
    """Round fp32 -> fp32r (11-bit mantissa, round-to-nearest-even)."""
    a = np.ascontiguousarray(a, np.float32)
    u = a.view(np.uint32).astype(np.uint64)
    drop = np.uint64(12)
    one = np.uint64(1)
    half = np.uint64(1 << 11)
    mask = ~np.uint64((1 << 12) - 1)
    r = (u + half - one + ((u >> drop) & one)) & mask
    return r.astype(np.uint32).view(np.float32)


def _to_bf16(a):
    return np.asarray(a).astype(_BF16)


def _poly_roots_host(x):
    """Exact copy of the reference poly_roots, forced onto CPU jax."""
    import jax
    cpu = jax.devices("cpu")[0]
    with jax.default_device(cpu):
        import jax.numpy as jnp
        xj = jax.device_put(np.asarray(x), cpu)
        coeffs = jax.lax.complex(xj[..., 0], xj[..., 1])
        b = coeffs.shape[0]
        norm = coeffs / coeffs[:, :1]
        c = -jnp.flip(norm[:, 1:], axis=-1)
        C = jnp.broadcast_to(jnp.eye(K, k=-1, dtype=coeffs.dtype), (b, K, K))
        C = C.at[:, :, -1].set(c)
        eigs = jnp.linalg.eigvals(C)
        out = jnp.stack([eigs.real, eigs.imag], axis=-1).reshape(b, 2 * K)
        return np.asarray(out.astype(jnp.float32))


def _fold_weights(bw, sw, sc):
    """[O,I] base + [O,I,8]*[O,I] spline weights -> [O, I, 10] folded:
    col 0 = base weight (silu feature), cols 1..9 = (W_j - W_{j-1})/6."""
    W = sw.astype(np.float64) * sc.astype(np.float64)[..., None]        # [O,I,8]
    O, I = W.shape[:2]
    Wext = np.zeros((O, I, 10))
    Wext[:, :, 1:9] = W
    wp = (Wext[:, :, 1:10] - Wext[:, :, 0:9]) / 6.0                      # [O,I,9]
    return np.concatenate([bw.astype(np.float64)[:, :, None], wp], axis=2)


# ---------------------------------------------------------------- custom DVE op
_CAPCUBE = None


def _get_capcube():
    """out = min(relu(in0*imm2 + s0), s1)^3  — one DVE pass."""
    global _CAPCUBE
    if _CAPCUBE is not None:
        return _CAPCUBE
    from concourse.dve_spec import Spec, Src0, C0, C1, C2, minn, relu, sq, lower
    from concourse import dve_ops
    from concourse.dve_uop import DveOpSpec

    name = "KAN_CAPCUBE_ANT"
    if name in dve_ops._SUB_OPCODE_FOR_NAME:
        _CAPCUBE = next(op for op in dve_ops.OPS if op.name == name)
        return _CAPCUBE

    def _ref(in0, in1, s0, s1, imm2):
        m = np.minimum(np.maximum(in0 * imm2 + s0, 0.0), s1)
        return (m * m * m).astype(np.float32)

    m = minn(relu(Src0 * C2 + C0), C1)
    spec = Spec(body=sq(m) * m, reference=_ref)
    shas = {}
    for ver in ("v3", "v4"):
        try:
            s = DveOpSpec(name=name, opcode=1, uops=lower(spec, ver=ver), rd1_en=False)
            shas[ver] = s.sha(ver)
        except Exception:
            pass
    op = dve_ops.DveOp(name, spec, subdim=False, uops_sha=shas)
    dve_ops.OPS.append(op)
    dve_ops.CUSTOM_DVE_SPECS[name] = spec
    dve_ops._SUB_OPCODE_FOR_NAME[name] = dve_ops._CUSTOM_DVE_ROW_BASE + len(dve_ops.OPS) - 1
    _CAPCUBE = op
    return op


# ---------------------------------------------------------------- bass program
_PROGRAMS = {}

# combine-op engine assignment per j-group g=0,1,2: 'v' = DVE (stt form),
# 'p' = Pool (plain tensor_tensor only — walrus rejects stt on Pool).
# q (= 3*a2 + a1, stt) always runs on DVE.
A1_ENG = ('p', 'p', 'p')
A2_ENG = ('p', 'v', 'v')
A1_ENG0 = ('p', 'p', 'p')
A2_ENG0 = ('p', 'v', 'v')


def build_program(iters=1):
    """Build (and cache) the compiled per-core Bass program."""
    if iters in _PROGRAMS:
        return _PROGRAMS[iters]

    import concourse.bacc as bacc
    import concourse.bass as bass
    import concourse.tile as tile
    import concourse.mybir as mybir

    F32 = mybir.dt.float32
    F32R = mybir.dt.float32r
    BF16 = mybir.dt.bfloat16
    AFT = mybir.ActivationFunctionType
    ALU = mybir.AluOpType
    CAPCUBE = _get_capcube()

    nc = bacc.Bacc("TRN2", target_bir_lowering=False, debug=False)

    rt_d = nc.dram_tensor("rt", [IN0, BC], F32, kind="ExternalInput")
    w0_d = nc.dram_tensor("w0t", [C0_ROWS, HID], F32R, kind="ExternalInput")
    w1_d = nc.dram_tensor("w1t", [C1_TILES, 128, OUT], BF16, kind="ExternalInput")
    out_d = nc.dram_tensor("out", [OUT, BC], F32, kind="ExternalOutput")

    def sub_tt(which, out_ap, a_ap, b_ap):
        """out = a - b on Pool (tensor_tensor) or DVE (stt form, faster)."""
        if which == 'p':
            nc.gpsimd.tensor_sub(out_ap, a_ap, b_ap)
        else:
            nc.vector.scalar_tensor_tensor(out_ap, a_ap, 1.0, b_ap,
                                           ALU.mult, ALU.subtract)

    with tile.TileContext(nc) as tc:
        with ExitStack() as ctx:
            const = ctx.enter_context(tc.tile_pool(name="const", bufs=2))
            wconst = ctx.enter_context(tc.tile_pool(name="wconst", bufs=1))
            f0p = ctx.enter_context(tc.tile_pool(name="f0p", bufs=2))
            hp = ctx.enter_context(tc.tile_pool(name="hp", bufs=8))
            phip = ctx.enter_context(tc.tile_pool(name="phip", bufs=1))
            chp = ctx.enter_context(tc.tile_pool(name="chp", bufs=3))
            qtp = ctx.enter_context(tc.tile_pool(name="qtp", bufs=2))
            silp = ctx.enter_context(tc.tile_pool(name="silp", bufs=3))
            w1p = ctx.enter_context(tc.tile_pool(name="w1p", bufs=10))
            osp = ctx.enter_context(tc.tile_pool(name="osp", bufs=4))
            psp = ctx.enter_context(tc.tile_pool(name="psp", bufs=8, space="PSUM"))

            loop_cm = (tc.For_i(0, iters, 1, hint_engines=(
                mybir.EngineType.PE, mybir.EngineType.DVE,
                mybir.EngineType.Activation, mybir.EngineType.Pool,
                mybir.EngineType.SP)) if iters > 1 else None)
            if loop_cm is not None:
                loop_cm.__enter__()

            # ---------------- layer 0 ----------------
            # roots packed as 4 batch-blocks of 256 on partition quarters
            BB = BC // 4   # 256
            rt4 = const.tile([128, BB], F32, name="rt4")
            for k in range(4):
                nc.sync.dma_start(rt4[32 * k:32 * k + IN0, :],
                                  rt_d.ap()[:, k * BB:(k + 1) * BB])

            f0 = [
                f0p.tile([128, BC], F32R, name="f0a"),
                f0p.tile([128, BC], F32R, name="f0b"),
                f0p.tile([64, BC], F32R, name="f0c"),
            ]
            # zero-fill (pad rows must be finite)
            zt = wconst.tile([128, BC], F32, name="zt")
            nc.vector.memset(zt[:], 0.0)
            for t in f0:
                p = t.shape[0]
                nc.scalar.copy(t[:], zt[0:p, :])

            # silu section (s=0) -> rows 0..19 of f0[0]
            sil0 = silp.tile([128, BB], F32, name="sil0")
            nc.scalar.activation(sil0[:], rt4[:], AFT.Silu)
            for k in range(4):
                nc.scalar.copy(f0[0][0:IN0, k * BB:(k + 1) * BB],
                               sil0[32 * k:32 * k + IN0, :])

            phi0 = phip.tile([128, NR * BB], F32, name="phi0")
            for r in range(NR):
                nc.vector._custom_dve(
                    CAPCUBE,
                    out=phi0[:, r * BB:(r + 1) * BB],
                    in0=rt4[:],
                    s0=float(5.5 - r),
                    s1=float(11 - r),
                    imm2=2.5,
                )
            for g in range(3):  # j-groups of 3
                gw = 3 * BB

                def ph0(r0):
                    return phi0[:, r0 * BB:r0 * BB + gw]

                j0 = 3 * g
                a1 = chp.tile([128, gw], F32, name="ch0")
                sub_tt(A1_ENG0[g], a1[:], ph0(j0), ph0(j0 + 3))
                a2 = chp.tile([128, gw], F32, name="ch0")
                sub_tt(A2_ENG0[g], a2[:], ph0(j0 + 2), ph0(j0 + 1))
                q0 = chp.tile([128, gw], F32, name="ch0")
                nc.vector.scalar_tensor_tensor(
                    q0[:], a2[:], 3.0, a1[:], ALU.mult, ALU.add)
                for jj in range(3):
                    j = j0 + jj
                    row = 32 * (1 + j)
                    ti, off = row // 128, row % 128
                    for k in range(4):
                        nc.scalar.copy(
                            f0[ti][off:off + IN0, k * BB:(k + 1) * BB],
                            q0[32 * k:32 * k + IN0, jj * BB:(jj + 1) * BB])

            w0 = [
                wconst.tile([128, HID], F32R, name="w0a"),
                wconst.tile([128, HID], F32R, name="w0b"),
                wconst.tile([64, HID], F32R, name="w0c"),
            ]
            # split w0 DMA into 32-row chunks to spread across DMA engines
            for t, base in ((w0[0], 0), (w0[1], 128)):
                for k in range(4):
                    nc.sync.dma_start(t[32 * k:32 * (k + 1), :],
                                      w0_d.ap()[base + 32 * k:base + 32 * (k + 1), :])
            for k in range(2):
                nc.sync.dma_start(w0[2][32 * k:32 * (k + 1), :],
                                  w0_d.ap()[256 + 32 * k:256 + 32 * (k + 1), :])

            hT = [hp.tile([128, BC], F32, name="hT") for _ in range(8)]
            for bh in range(2):
                bsl = slice(bh * 512, (bh + 1) * 512)
                for o in range(8):
                    ps = psp.tile([128, 512], F32, name="ps")
                    for t in range(3):
                        nc.tensor.matmul(
                            ps[:], w0[t][:, o * 128:(o + 1) * 128], f0[t][:, bsl],
                            start=(t == 0), stop=(t == 2))
                    nc.scalar.copy(hT[o][:, bsl], ps[:])

            # ---------------- layer 1 ----------------
            for ch in range(2):
                bsl = slice(ch * 512, (ch + 1) * 512)
                pss = [psp.tile([128, 512], F32, name="ps") for _ in range(8)]
                for it in range(8):
                    x = hT[it][:, bsl]
                    sil = silp.tile([128, 512], BF16, name="sil")
                    nc.scalar.activation(sil[:], x, AFT.Silu)
                    phi = phip.tile([128, NR * 512], F32, name="phi")
                    for r in range(NR):
                        nc.vector._custom_dve(
                            CAPCUBE,
                            out=phi[:, r * 512:(r + 1) * 512],
                            in0=x,
                            s0=float(5.5 - r),
                            s1=float(11 - r),
                            imm2=2.5,
                        )
                    qt = qtp.tile([128, NJ * 512], BF16, name="qt")
                    for g in range(3):
                        gw = 3 * 512

                        def ph(r0):
                            return phi[:, r0 * 512:r0 * 512 + gw]

                        j0 = 3 * g
                        a1 = chp.tile([128, gw], F32, name="ch")
                        sub_tt(A1_ENG[g], a1[:], ph(j0), ph(j0 + 3))
                        a2 = chp.tile([128, gw], F32, name="ch")
                        sub_tt(A2_ENG[g], a2[:], ph(j0 + 2), ph(j0 + 1))
                        # stt with bf16 out runs 2.1x slower on DVE; keep the
                        # combine in f32 and convert on the idle ACT engine
                        qf = chp.tile([128, gw], F32, name="ch")
                        nc.vector.scalar_tensor_tensor(
                            qf[:], a2[:], 3.0, a1[:], ALU.mult, ALU.add)
                        nc.scalar.copy(qt[:, j0 * 512:j0 * 512 + gw], qf[:])
                    for s in range(NSEC):
                        c = it * NSEC + s
                        w = w1p.tile([128, OUT], BF16, name="w1")
                        nc.sync.dma_start(w[:], w1_d.ap()[c, :, :])
                        F = sil[:] if s == 0 else qt[:, (s - 1) * 512:s * 512]
                        for o in range(8):
                            nc.tensor.matmul(
                                pss[o][:], w[:, o * 128:(o + 1) * 128], F,
                                start=(c == 0), stop=(c == C1_TILES - 1))
                for o in range(8):
                    st = osp.tile([128, 512], F32, name="ost")
                    nc.scalar.copy(st[:], pss[o][:])
                    nc.sync.dma_start(out_d.ap()[o * 128:(o + 1) * 128, bsl], st[:])

            if loop_cm is not None:
                loop_cm.__exit__(None, None, None)

    nc.compile()
    _PROGRAMS[iters] = nc
    return nc


# ---------------------------------------------------------------- host driver
def prepare_inputs(x, bw0, sw0, sc0, bw1, sw1, sc1):
    """Host-side prep: roots + folded weights + per-core in_maps."""
    roots = _poly_roots_host(np.asarray(x, np.float32))          # [B, 20]
    rootsT = np.ascontiguousarray(roots.T)                        # [20, B]

    W0 = _fold_weights(bw0, sw0, sc0)                             # [1024, 20, 10]
    W1 = _fold_weights(bw1, sw1, sc1)                             # [1024, 1024, 10]

    # layer-0 DRAM layout [320, 1024] f32r: row 32*s + i -> W0[o, i, s]
    w0t = np.zeros((C0_ROWS, HID), np.float64)
    for s in range(NSEC):
        w0t[32 * s:32 * s + IN0, :] = W0[:, :, s].T
    w0t = _round_f32r(w0t.astype(np.float32))

    # layer-1 DRAM layout [80, 128, 1024] bf16: (c = it*10 + s, p, o)
    w1t = np.empty((C1_TILES, 128, OUT), _BF16)
    for it in range(8):
        blk = W1[:, it * 128:(it + 1) * 128, :]                   # [O, 128, 10]
        for s in range(NSEC):
            w1t[it * NSEC + s] = blk[:, :, s].T.astype(_BF16)

    in_maps = []
    for c in range(CORES):
        in_maps.append({
            "rt": np.ascontiguousarray(rootsT[:, c * BC:(c + 1) * BC]),
            "w0t": w0t,
            "w1t": w1t,
        })
    return in_maps


def assemble_output(results):
    """Per-core [OUT, BC] (o, b) outputs -> full [B, OUT]."""
    return np.ascontiguousarray(
        np.concatenate([np.asarray(r["out"]).T for r in results], axis=0)
    ).astype(np.float32)


def kernel(x, bw0, sw0, sc0, bw1, sw1, sc1):
    from concourse.bass_utils import run_bass_kernel_spmd
    args = [np.asarray(a, np.float32) for a in (x, bw0, sw0, sc0, bw1, sw1, sc1)]
    nc = build_program()
    in_maps = prepare_inputs(*args)
    res = run_bass_kernel_spmd(nc, in_maps, list(range(CORES)))
    return assemble_output(res.results)


# revision 15
# speedup vs baseline: 1.0269x; 1.0269x over previous
"""Trainium2 kernel for nn_Decoder_70781061038948.

Pipeline: poly_roots (companion-matrix eigvals) -> KAN layer (20->1024) ->
KAN layer (1024->1024).

Device strategy (8 NeuronCores, pure data-parallel over the 8192 batch):
  * Eigenvalues on host CPU with the exact jax/LAPACK call the reference
    uses (LAPACK's eigenvalue ORDER is not reproducible on device, and the
    KAN layers are order-sensitive). ~0% of FLOPs and bytes.
  * Everything else (B-spline features + both KAN layers) runs on the 8
    cores, batch-sharded 1024/core.

Math: uniform-knot cubic B-splines rewritten through the cumulative basis
  6*Qt_j(x) = phi_j - 3 phi_{j+1} + 3 phi_{j+2} - phi_{j+3},
  phi_r = min(relu(2.5x + 5.5 - r), 11 - r)^3
so each KAN layer is ONE dense matmul over 10 features per input
(silu + 9 bounded Qt). Weight differencing and the 1/6 are folded into the
weights on host in float64.

v2 (from NTFF profile of v1):
  * v1 span 443us/iter: DVE 89% busy (slow 2-stream tensor_tensor +
    gpsimd subs), PE 78%, weight DMA 2x42MB/iter at the ~310GB/s DMA cap.
  * Layer-1 weights+features now bf16 (PE rate unchanged, DMA and SBUF
    halved; adds ~2e-3 rel err vs 2e-2 budget).  Layer-0 stays f32r so h
    (the layer-1 spline argument) keeps full precision.
  * Feature combines are scalar_tensor_tensor only (measured 2.2x faster
    than tensor_tensor on DVE), split across DVE and Pool:
       u_g = 3 phi_{j+2} - phi_{j+3};  v_g = phi_j - 3 phi_{j+1}
       qt_g = u_g + v_g   (bf16 out)
  * Deeper weight prefetch (w1p bufs), w0 DMA split into 32-row chunks.
"""

import numpy as np
from contextlib import ExitStack

# ---------------------------------------------------------------- constants
K = 10
B = 8192
CORES = 8
BC = B // CORES            # 1024 batch rows per core
IN0 = 2 * K                # 20
HID = 1024
OUT = 1024
NJ = 9                     # Qt features per input
NSEC = 1 + NJ              # silu + 9 Qt
NR = 12                    # relu-cube shifts r = 0..11
C0_ROWS = 32 * NSEC        # 320 (layer-0 c-layout: 10 sections of 32 rows)
C1_TILES = 8 * NSEC        # 80 (layer-1 c-tiles of 128)

_f32 = np.float32

try:
    import ml_dtypes
    _BF16 = ml_dtypes.bfloat16
except ImportError:  # pragma: no cover
    _BF16 = None


def _round_f32r(a):
    """Round fp32 -> fp32r (11-bit mantissa, round-to-nearest-even)."""
    a = np.ascontiguousarray(a, np.float32)
    u = a.view(np.uint32).astype(np.uint64)
    drop = np.uint64(12)
    one = np.uint64(1)
    half = np.uint64(1 << 11)
    mask = ~np.uint64((1 << 12) - 1)
    r = (u + half - one + ((u >> drop) & one)) & mask
    return r.astype(np.uint32).view(np.float32)


def _to_bf16(a):
    return np.asarray(a).astype(_BF16)


def _poly_roots_host(x):
    """Exact copy of the reference poly_roots, forced onto CPU jax."""
    import jax
    cpu = jax.devices("cpu")[0]
    with jax.default_device(cpu):
        import jax.numpy as jnp
        xj = jax.device_put(np.asarray(x), cpu)
        coeffs = jax.lax.complex(xj[..., 0], xj[..., 1])
        b = coeffs.shape[0]
        norm = coeffs / coeffs[:, :1]
        c = -jnp.flip(norm[:, 1:], axis=-1)
        C = jnp.broadcast_to(jnp.eye(K, k=-1, dtype=coeffs.dtype), (b, K, K))
        C = C.at[:, :, -1].set(c)
        eigs = jnp.linalg.eigvals(C)
        out = jnp.stack([eigs.real, eigs.imag], axis=-1).reshape(b, 2 * K)
        return np.asarray(out.astype(jnp.float32))


def _fold_weights(bw, sw, sc):
    """[O,I] base + [O,I,8]*[O,I] spline weights -> [O, I, 10] folded:
    col 0 = base weight (silu feature), cols 1..9 = (W_j - W_{j-1})/6."""
    W = sw.astype(np.float64) * sc.astype(np.float64)[..., None]        # [O,I,8]
    O, I = W.shape[:2]
    Wext = np.zeros((O, I, 10))
    Wext[:, :, 1:9] = W
    wp = (Wext[:, :, 1:10] - Wext[:, :, 0:9]) / 6.0                      # [O,I,9]
    return np.concatenate([bw.astype(np.float64)[:, :, None], wp], axis=2)


# ---------------------------------------------------------------- custom DVE op
_CAPCUBE = None


def _get_capcube():
    """out = min(relu(in0*imm2 + s0), s1)^3  — one DVE pass."""
    global _CAPCUBE
    if _CAPCUBE is not None:
        return _CAPCUBE
    from concourse.dve_spec import Spec, Src0, C0, C1, C2, minn, relu, sq, lower
    from concourse import dve_ops
    from concourse.dve_uop import DveOpSpec

    name = "KAN_CAPCUBE_ANT"
    if name in dve_ops._SUB_OPCODE_FOR_NAME:
        _CAPCUBE = next(op for op in dve_ops.OPS if op.name == name)
        return _CAPCUBE

    def _ref(in0, in1, s0, s1, imm2):
        m = np.minimum(np.maximum(in0 * imm2 + s0, 0.0), s1)
        return (m * m * m).astype(np.float32)

    m = minn(relu(Src0 * C2 + C0), C1)
    spec = Spec(body=sq(m) * m, reference=_ref)
    shas = {}
    for ver in ("v3", "v4"):
        try:
            s = DveOpSpec(name=name, opcode=1, uops=lower(spec, ver=ver), rd1_en=False)
            shas[ver] = s.sha(ver)
        except Exception:
            pass
    op = dve_ops.DveOp(name, spec, subdim=False, uops_sha=shas)
    dve_ops.OPS.append(op)
    dve_ops.CUSTOM_DVE_SPECS[name] = spec
    dve_ops._SUB_OPCODE_FOR_NAME[name] = dve_ops._CUSTOM_DVE_ROW_BASE + len(dve_ops.OPS) - 1
    _CAPCUBE = op
    return op


# ---------------------------------------------------------------- bass program
_PROGRAMS = {}

# combine-op engine assignment per j-group g=0,1,2: 'v' = DVE (stt form),
# 'p' = Pool (plain tensor_tensor only — walrus rejects stt on Pool).
# Group 0 is consumed FIRST by the PE, so it runs all-DVE (low latency);
# Pool (4us/op) takes the later groups.  q (stt) + cvt (ACT) always v/ACT.
A1_ENG = ('v', 'p', 'p')
A2_ENG = ('v', 'p', 'p')
# loop unroll: For_i drains every engine at each iteration; unrolling the
# body halves that boundary cost. iters must be divisible (or 1).
UNROLL = 2


def build_program(iters=1):
    """Build (and cache) the compiled per-core Bass program."""
    if iters in _PROGRAMS:
        return _PROGRAMS[iters]

    import concourse.bacc as bacc
    import concourse.bass as bass
    import concourse.tile as tile
    import concourse.mybir as mybir

    F32 = mybir.dt.float32
    F32R = mybir.dt.float32r
    BF16 = mybir.dt.bfloat16
    AFT = mybir.ActivationFunctionType
    ALU = mybir.AluOpType
    CAPCUBE = _get_capcube()

    nc = bacc.Bacc("TRN2", target_bir_lowering=False, debug=False)

    rt_d = nc.dram_tensor("rt", [IN0, BC], F32, kind="ExternalInput")
    w0_d = nc.dram_tensor("w0t", [C0_ROWS, HID], F32R, kind="ExternalInput")
    w1_d = nc.dram_tensor("w1t", [C1_TILES, 128, OUT], BF16, kind="ExternalInput")
    out_d = nc.dram_tensor("out", [OUT, BC], F32, kind="ExternalOutput")

    def sub_tt(which, out_ap, a_ap, b_ap):
        """out = a - b on Pool (tensor_tensor) or DVE (stt form, faster)."""
        if which == 'p':
            nc.gpsimd.tensor_sub(out_ap, a_ap, b_ap)
        else:
            nc.vector.scalar_tensor_tensor(out_ap, a_ap, 1.0, b_ap,
                                           ALU.mult, ALU.subtract)

    unroll = UNROLL if iters > 1 else 1
    assert iters == 1 or iters % unroll == 0

    with tile.TileContext(nc) as tc:
        with ExitStack() as ctx:
            const = ctx.enter_context(tc.tile_pool(name="const", bufs=2))
            wconst = ctx.enter_context(tc.tile_pool(name="wconst", bufs=1))
            f0p = ctx.enter_context(tc.tile_pool(name="f0p", bufs=1))
            hp = ctx.enter_context(tc.tile_pool(name="hp", bufs=8))
            phip = ctx.enter_context(tc.tile_pool(name="phip", bufs=1))
            chp = ctx.enter_context(tc.tile_pool(name="chp", bufs=4))
            qfp = ctx.enter_context(tc.tile_pool(name="qfp", bufs=2))
            qtp = ctx.enter_context(tc.tile_pool(name="qtp", bufs=2))
            silp = ctx.enter_context(tc.tile_pool(name="silp", bufs=2))
            w1p = ctx.enter_context(tc.tile_pool(name="w1p", bufs=10))
            osp = ctx.enter_context(tc.tile_pool(name="osp", bufs=4))
            psp = ctx.enter_context(tc.tile_pool(name="psp", bufs=8, space="PSUM"))

            def emit_features(x, width, qt_out, cvt):
                """Interleaved capcube + combine emission for one input tile.

                x: [128, width] input AP; qt_out(j0_gw) -> dest AP per group;
                cvt: 'act' = q via f32 staging + ACT convert into dest;
                None = stt writes dest directly (f32 dest).
                Tiles are allocated at layer-1 size with views for width<512
                so L0/L1 share pool slots."""
                phi = phip.tile([128, NR * 512], F32, name="phi")

                def cc(r):
                    nc.vector._custom_dve(
                        CAPCUBE, out=phi[:, r * width:(r + 1) * width], in0=x,
                        s0=float(5.5 - r), s1=float(11 - r), imm2=2.5)

                gw = 3 * width

                def ph(r0):
                    return phi[:, r0 * width:r0 * width + gw]

                def a_op(which, g, hi, lo):
                    t = chp.tile([128, 3 * 512], F32, name="ch")
                    sub_tt(which, t[:, 0:gw], ph(3 * g + hi), ph(3 * g + lo))
                    return t

                def q_op(g, a1, a2):
                    dst = qt_out(g, gw)
                    if cvt == 'act':
                        qf = qfp.tile([128, 3 * 512], F32, name="qf")
                        nc.vector.scalar_tensor_tensor(
                            qf[:, 0:gw], a2[:, 0:gw], 3.0, a1[:, 0:gw],
                            ALU.mult, ALU.add)
                        nc.scalar.copy(dst, qf[:, 0:gw])
                    else:
                        nc.vector.scalar_tensor_tensor(
                            dst, a2[:, 0:gw], 3.0, a1[:, 0:gw],
                            ALU.mult, ALU.add)

                for r in range(6):          # cc0..cc5
                    cc(r)
                a1_0 = a_op(A1_ENG[0], 0, 0, 3)
                a2_0 = a_op(A2_ENG[0], 0, 2, 1)
                q_op(0, a1_0, a2_0)
                cc(6)
                cc(7)
                a2_1 = a_op(A2_ENG[1], 1, 2, 1)   # needs phi4..7
                cc(8)
                a1_1 = a_op(A1_ENG[1], 1, 0, 3)   # needs phi3..8
                cc(9)
                cc(10)
                a2_2 = a_op(A2_ENG[2], 2, 2, 1)   # needs phi7..10
                cc(11)
                a1_2 = a_op(A1_ENG[2], 2, 0, 3)   # needs phi6..11
                q_op(1, a1_1, a2_1)
                q_op(2, a1_2, a2_2)

            def emit_body():
                # ---------------- layer 0 ----------------
                BB = BC // 4   # 256; 4 batch-blocks on partition quarters
                rt4 = const.tile([128, BB], F32, name="rt4")
                for k in range(4):
                    nc.sync.dma_start(rt4[32 * k:32 * k + IN0, :],
                                      rt_d.ap()[:, k * BB:(k + 1) * BB])

                f0 = [
                    f0p.tile([128, BC], F32R, name="f0a"),
                    f0p.tile([128, BC], F32R, name="f0b"),
                    f0p.tile([64, BC], F32R, name="f0c"),
                ]
                # pad rows must be finite; ACT copy is an f32r-rounding
                # producer (BIR verifier requires one for f32r matmul inputs)
                zt = wconst.tile([128, BC], F32, name="zt")
                nc.vector.memset(zt[:], 0.0)
                for t in f0:
                    p = t.shape[0]
                    nc.scalar.copy(t[:], zt[0:p, :])

                sil0 = silp.tile([128, BB], F32, name="sil0")
                nc.scalar.activation(sil0[:], rt4[:], AFT.Silu)
                for k in range(4):
                    nc.scalar.copy(f0[0][0:IN0, k * BB:(k + 1) * BB],
                                   sil0[32 * k:32 * k + IN0, :])

                # features -> per-group q staging tiles, then ACT copies
                q0_tiles = {}

                def qt_out0(g, gw):
                    t = qfp.tile([128, 3 * 512], F32, name="qf")
                    q0_tiles[g] = t
                    return t[:, 0:gw]

                emit_features(rt4[:], BB, qt_out0, cvt=None)
                for g in range(3):
                    q0 = q0_tiles[g]
                    for jj in range(3):
                        j = 3 * g + jj
                        row = 32 * (1 + j)
                        ti, off = row // 128, row % 128
                        for k in range(4):
                            nc.scalar.copy(
                                f0[ti][off:off + IN0, k * BB:(k + 1) * BB],
                                q0[32 * k:32 * k + IN0,
                                   jj * BB:(jj + 1) * BB])

                w0 = [
                    wconst.tile([128, HID], F32R, name="w0a"),
                    wconst.tile([128, HID], F32R, name="w0b"),
                    wconst.tile([64, HID], F32R, name="w0c"),
                ]
                # split w0 DMA into 32-row chunks to spread across DMA engines
                for t, base in ((w0[0], 0), (w0[1], 128)):
                    for k in range(4):
                        nc.sync.dma_start(
                            t[32 * k:32 * (k + 1), :],
                            w0_d.ap()[base + 32 * k:base + 32 * (k + 1), :])
                for k in range(2):
                    nc.sync.dma_start(
                        w0[2][32 * k:32 * (k + 1), :],
                        w0_d.ap()[256 + 32 * k:256 + 32 * (k + 1), :])

                hT = [hp.tile([128, BC], F32, name="hT") for _ in range(8)]
                for bh in range(2):
                    bsl = slice(bh * 512, (bh + 1) * 512)
                    for o in range(8):
                        ps = psp.tile([128, 512], F32, name="ps")
                        for t in range(3):
                            nc.tensor.matmul(
                                ps[:], w0[t][:, o * 128:(o + 1) * 128],
                                f0[t][:, bsl], start=(t == 0), stop=(t == 2))
                        nc.scalar.copy(hT[o][:, bsl], ps[:])

                # ---------------- layer 1 ----------------
                for ch in range(2):
                    bsl = slice(ch * 512, (ch + 1) * 512)
                    pss = [psp.tile([128, 512], F32, name="ps")
                           for _ in range(8)]
                    for it in range(8):
                        x = hT[it][:, bsl]
                        sil = silp.tile([128, 512], BF16, name="sil")
                        nc.scalar.activation(sil[:], x, AFT.Silu)
                        qt = qtp.tile([128, NJ * 512], BF16, name="qt")

                        def qt_out(g, gw):
                            return qt[:, 3 * g * 512:3 * g * 512 + gw]

                        emit_features(x, 512, qt_out, cvt='act')
                        for s in range(NSEC):
                            c = it * NSEC + s
                            w = w1p.tile([128, OUT], BF16, name="w1")
                            nc.sync.dma_start(w[:], w1_d.ap()[c, :, :])
                            F = sil[:] if s == 0 else qt[:, (s - 1) * 512:s * 512]
                            for o in range(8):
                                nc.tensor.matmul(
                                    pss[o][:], w[:, o * 128:(o + 1) * 128], F,
                                    start=(c == 0), stop=(c == C1_TILES - 1))
                    for o in range(8):
                        st = osp.tile([128, 512], F32, name="ost")
                        nc.scalar.copy(st[:], pss[o][:])
                        nc.sync.dma_start(
                            out_d.ap()[o * 128:(o + 1) * 128, bsl], st[:])

            loop_cm = (tc.For_i(0, iters // unroll, 1, hint_engines=(
                mybir.EngineType.PE, mybir.EngineType.DVE,
                mybir.EngineType.Activation, mybir.EngineType.Pool,
                mybir.EngineType.SP)) if iters > 1 else None)
            if loop_cm is not None:
                loop_cm.__enter__()
            for _ in range(unroll):
                emit_body()
            if loop_cm is not None:
                loop_cm.__exit__(None, None, None)

    nc.compile()
    _PROGRAMS[iters] = nc
    return nc


# ---------------------------------------------------------------- host driver
def prepare_inputs(x, bw0, sw0, sc0, bw1, sw1, sc1):
    """Host-side prep: roots + folded weights + per-core in_maps."""
    roots = _poly_roots_host(np.asarray(x, np.float32))          # [B, 20]
    rootsT = np.ascontiguousarray(roots.T)                        # [20, B]

    W0 = _fold_weights(bw0, sw0, sc0)                             # [1024, 20, 10]
    W1 = _fold_weights(bw1, sw1, sc1)                             # [1024, 1024, 10]

    # layer-0 DRAM layout [320, 1024] f32r: row 32*s + i -> W0[o, i, s]
    w0t = np.zeros((C0_ROWS, HID), np.float64)
    for s in range(NSEC):
        w0t[32 * s:32 * s + IN0, :] = W0[:, :, s].T
    w0t = _round_f32r(w0t.astype(np.float32))

    # layer-1 DRAM layout [80, 128, 1024] bf16: (c = it*10 + s, p, o)
    w1t = np.empty((C1_TILES, 128, OUT), _BF16)
    for it in range(8):
        blk = W1[:, it * 128:(it + 1) * 128, :]                   # [O, 128, 10]
        for s in range(NSEC):
            w1t[it * NSEC + s] = blk[:, :, s].T.astype(_BF16)

    in_maps = []
    for c in range(CORES):
        in_maps.append({
            "rt": np.ascontiguousarray(rootsT[:, c * BC:(c + 1) * BC]),
            "w0t": w0t,
            "w1t": w1t,
        })
    return in_maps


def assemble_output(results):
    """Per-core [OUT, BC] (o, b) outputs -> full [B, OUT]."""
    return np.ascontiguousarray(
        np.concatenate([np.asarray(r["out"]).T for r in results], axis=0)
    ).astype(np.float32)


def kernel(x, bw0, sw0, sc0, bw1, sw1, sc1):
    from concourse.bass_utils import run_bass_kernel_spmd
    args = [np.asarray(a, np.float32) for a in (x, bw0, sw0, sc0, bw1, sw1, sc1)]
    nc = build_program()
    in_maps = prepare_inputs(*args)
    res = run_bass_kernel_spmd(nc, in_maps, list(range(CORES)))
    return assemble_output(res.results)


# revision 16
# speedup vs baseline: 1.0497x; 1.0222x over previous
"""Trainium2 kernel for nn_Decoder_70781061038948.

Pipeline: poly_roots (companion-matrix eigvals) -> KAN layer (20->1024) ->
KAN layer (1024->1024).

Device strategy (8 NeuronCores, pure data-parallel over the 8192 batch):
  * Eigenvalues on host CPU with the exact jax/LAPACK call the reference
    uses (LAPACK's eigenvalue ORDER is not reproducible on device, and the
    KAN layers are order-sensitive). ~0% of FLOPs and bytes.
  * Everything else (B-spline features + both KAN layers) runs on the 8
    cores, batch-sharded 1024/core.

Math: uniform-knot cubic B-splines rewritten through the cumulative basis
  6*Qt_j(x) = phi_j - 3 phi_{j+1} + 3 phi_{j+2} - phi_{j+3},
  phi_r = min(relu(2.5x + 5.5 - r), 11 - r)^3
so each KAN layer is ONE dense matmul over 10 features per input
(silu + 9 bounded Qt). Weight differencing and the 1/6 are folded into the
weights on host in float64.

v2 (from NTFF profile of v1):
  * v1 span 443us/iter: DVE 89% busy (slow 2-stream tensor_tensor +
    gpsimd subs), PE 78%, weight DMA 2x42MB/iter at the ~310GB/s DMA cap.
  * Layer-1 weights+features now bf16 (PE rate unchanged, DMA and SBUF
    halved; adds ~2e-3 rel err vs 2e-2 budget).  Layer-0 stays f32r so h
    (the layer-1 spline argument) keeps full precision.
  * Feature combines are scalar_tensor_tensor only (measured 2.2x faster
    than tensor_tensor on DVE), split across DVE and Pool:
       u_g = 3 phi_{j+2} - phi_{j+3};  v_g = phi_j - 3 phi_{j+1}
       qt_g = u_g + v_g   (bf16 out)
  * Deeper weight prefetch (w1p bufs), w0 DMA split into 32-row chunks.
"""

import numpy as np
from contextlib import ExitStack

# ---------------------------------------------------------------- constants
K = 10
B = 8192
CORES = 8
BC = B // CORES            # 1024 batch rows per core
IN0 = 2 * K                # 20
HID = 1024
OUT = 1024
NJ = 9                     # Qt features per input
NSEC = 1 + NJ              # silu + 9 Qt
NR = 12                    # relu-cube shifts r = 0..11
C0_ROWS = 32 * NSEC        # 320 (layer-0 c-layout: 10 sections of 32 rows)
C1_TILES = 8 * NSEC        # 80 (layer-1 c-tiles of 128)

_f32 = np.float32

try:
    import ml_dtypes
    _BF16 = ml_dtypes.bfloat16
except ImportError:  # pragma: no cover
    _BF16 = None


def _round_f32r(a):
    """Round fp32 -> fp32r (11-bit mantissa, round-to-nearest-even)."""
    a = np.ascontiguousarray(a, np.float32)
    u = a.view(np.uint32).astype(np.uint64)
    drop = np.uint64(12)
    one = np.uint64(1)
    half = np.uint64(1 << 11)
    mask = ~np.uint64((1 << 12) - 1)
    r = (u + half - one + ((u >> drop) & one)) & mask
    return r.astype(np.uint32).view(np.float32)


def _to_bf16(a):
    return np.asarray(a).astype(_BF16)


def _poly_roots_host(x):
    """Exact copy of the reference poly_roots, forced onto CPU jax."""
    import jax
    cpu = jax.devices("cpu")[0]
    with jax.default_device(cpu):
        import jax.numpy as jnp
        xj = jax.device_put(np.asarray(x), cpu)
        coeffs = jax.lax.complex(xj[..., 0], xj[..., 1])
        b = coeffs.shape[0]
        norm = coeffs / coeffs[:, :1]
        c = -jnp.flip(norm[:, 1:], axis=-1)
        C = jnp.broadcast_to(jnp.eye(K, k=-1, dtype=coeffs.dtype), (b, K, K))
        C = C.at[:, :, -1].set(c)
        eigs = jnp.linalg.eigvals(C)
        out = jnp.stack([eigs.real, eigs.imag], axis=-1).reshape(b, 2 * K)
        return np.asarray(out.astype(jnp.float32))


def _fold_weights(bw, sw, sc):
    """[O,I] base + [O,I,8]*[O,I] spline weights -> [O, I, 10] folded:
    col 0 = base weight (silu feature), cols 1..9 = (W_j - W_{j-1})/6."""
    W = sw.astype(np.float64) * sc.astype(np.float64)[..., None]        # [O,I,8]
    O, I = W.shape[:2]
    Wext = np.zeros((O, I, 10))
    Wext[:, :, 1:9] = W
    wp = (Wext[:, :, 1:10] - Wext[:, :, 0:9]) / 6.0                      # [O,I,9]
    return np.concatenate([bw.astype(np.float64)[:, :, None], wp], axis=2)


# ---------------------------------------------------------------- custom DVE op
_CAPCUBE = None


def _get_capcube():
    """out = min(relu(in0*imm2 + s0), s1)^3  — one DVE pass."""
    global _CAPCUBE
    if _CAPCUBE is not None:
        return _CAPCUBE
    from concourse.dve_spec import Spec, Src0, C0, C1, C2, minn, relu, sq, lower
    from concourse import dve_ops
    from concourse.dve_uop import DveOpSpec

    name = "KAN_CAPCUBE_ANT"
    if name in dve_ops._SUB_OPCODE_FOR_NAME:
        _CAPCUBE = next(op for op in dve_ops.OPS if op.name == name)
        return _CAPCUBE

    def _ref(in0, in1, s0, s1, imm2):
        m = np.minimum(np.maximum(in0 * imm2 + s0, 0.0), s1)
        return (m * m * m).astype(np.float32)

    m = minn(relu(Src0 * C2 + C0), C1)
    spec = Spec(body=sq(m) * m, reference=_ref)
    shas = {}
    for ver in ("v3", "v4"):
        try:
            s = DveOpSpec(name=name, opcode=1, uops=lower(spec, ver=ver), rd1_en=False)
            shas[ver] = s.sha(ver)
        except Exception:
            pass
    op = dve_ops.DveOp(name, spec, subdim=False, uops_sha=shas)
    dve_ops.OPS.append(op)
    dve_ops.CUSTOM_DVE_SPECS[name] = spec
    dve_ops._SUB_OPCODE_FOR_NAME[name] = dve_ops._CUSTOM_DVE_ROW_BASE + len(dve_ops.OPS) - 1
    _CAPCUBE = op
    return op


# ---------------------------------------------------------------- bass program
_PROGRAMS = {}

# combine-op engine assignment per j-group g=0,1,2: 'v' = DVE (stt form),
# 'p' = Pool (plain tensor_tensor only — walrus rejects stt on Pool).
# Group 0 is consumed FIRST by the PE, so it runs all-DVE (low latency);
# Pool (4us/op) takes the later groups.  q (stt) + cvt (ACT) always v/ACT.
A1_ENG = ('v', 'p', 'p')
A2_ENG = ('v', 'p', 'p')
# loop unroll: For_i drains every engine at each iteration; unrolling the
# body halves that boundary cost. iters must be divisible (or 1).
UNROLL = 2


def build_program(iters=1):
    """Build (and cache) the compiled per-core Bass program."""
    if iters in _PROGRAMS:
        return _PROGRAMS[iters]

    import concourse.bacc as bacc
    import concourse.bass as bass
    import concourse.tile as tile
    import concourse.mybir as mybir

    F32 = mybir.dt.float32
    F32R = mybir.dt.float32r
    BF16 = mybir.dt.bfloat16
    AFT = mybir.ActivationFunctionType
    ALU = mybir.AluOpType
    CAPCUBE = _get_capcube()

    nc = bacc.Bacc("TRN2", target_bir_lowering=False, debug=False)

    rt_d = nc.dram_tensor("rt", [IN0, BC], F32, kind="ExternalInput")
    w0_d = nc.dram_tensor("w0t", [C0_ROWS, HID], F32R, kind="ExternalInput")
    w1_d = nc.dram_tensor("w1t", [C1_TILES, 128, OUT], BF16, kind="ExternalInput")
    out_d = nc.dram_tensor("out", [OUT, BC], F32, kind="ExternalOutput")

    def sub_tt(which, out_ap, a_ap, b_ap):
        """out = a - b on Pool (tensor_tensor) or DVE (stt form, faster)."""
        if which == 'p':
            nc.gpsimd.tensor_sub(out_ap, a_ap, b_ap)
        else:
            nc.vector.scalar_tensor_tensor(out_ap, a_ap, 1.0, b_ap,
                                           ALU.mult, ALU.subtract)

    unroll = UNROLL if iters > 1 else 1
    assert iters == 1 or iters % unroll == 0

    with tile.TileContext(nc) as tc:
        with ExitStack() as ctx:
            const = ctx.enter_context(tc.tile_pool(name="const", bufs=2))
            wconst = ctx.enter_context(tc.tile_pool(name="wconst", bufs=1))
            f0p = ctx.enter_context(tc.tile_pool(name="f0p", bufs=1))
            hp = ctx.enter_context(tc.tile_pool(name="hp", bufs=8))
            phip = ctx.enter_context(tc.tile_pool(name="phip", bufs=2))
            chp = ctx.enter_context(tc.tile_pool(name="chp", bufs=4))
            qfp = ctx.enter_context(tc.tile_pool(name="qfp", bufs=2))
            qtp = ctx.enter_context(tc.tile_pool(name="qtp", bufs=2))
            silp = ctx.enter_context(tc.tile_pool(name="silp", bufs=2))
            w1p = ctx.enter_context(tc.tile_pool(name="w1p", bufs=10))
            osp = ctx.enter_context(tc.tile_pool(name="osp", bufs=2))
            psp = ctx.enter_context(tc.tile_pool(name="psp", bufs=8, space="PSUM"))

            def emit_features(x, width, qt_out, cvt):
                """Interleaved capcube + combine emission for one input tile.

                x: [128, width] input AP; qt_out(j0_gw) -> dest AP per group;
                cvt: 'act' = q via f32 staging + ACT convert into dest;
                None = stt writes dest directly (f32 dest).
                Tiles are allocated at layer-1 size with views for width<512
                so L0/L1 share pool slots."""
                phi = phip.tile([128, NR * 512], F32, name="phi")

                def cc(r):
                    nc.vector._custom_dve(
                        CAPCUBE, out=phi[:, r * width:(r + 1) * width], in0=x,
                        s0=float(5.5 - r), s1=float(11 - r), imm2=2.5)

                gw = 3 * width

                def ph(r0):
                    return phi[:, r0 * width:r0 * width + gw]

                def a_op(which, g, hi, lo):
                    t = chp.tile([128, 3 * 512], F32, name="ch")
                    sub_tt(which, t[:, 0:gw], ph(3 * g + hi), ph(3 * g + lo))
                    return t

                def q_op(g, a1, a2):
                    dst = qt_out(g, gw)
                    if cvt == 'act':
                        qf = qfp.tile([128, 3 * 512], F32, name="qf")
                        nc.vector.scalar_tensor_tensor(
                            qf[:, 0:gw], a2[:, 0:gw], 3.0, a1[:, 0:gw],
                            ALU.mult, ALU.add)
                        nc.scalar.copy(dst, qf[:, 0:gw])
                    else:
                        nc.vector.scalar_tensor_tensor(
                            dst, a2[:, 0:gw], 3.0, a1[:, 0:gw],
                            ALU.mult, ALU.add)

                for r in range(6):          # cc0..cc5
                    cc(r)
                a1_0 = a_op(A1_ENG[0], 0, 0, 3)
                a2_0 = a_op(A2_ENG[0], 0, 2, 1)
                q_op(0, a1_0, a2_0)
                cc(6)
                cc(7)
                a2_1 = a_op(A2_ENG[1], 1, 2, 1)   # needs phi4..7
                cc(8)
                a1_1 = a_op(A1_ENG[1], 1, 0, 3)   # needs phi3..8
                cc(9)
                cc(10)
                a2_2 = a_op(A2_ENG[2], 2, 2, 1)   # needs phi7..10
                cc(11)
                a1_2 = a_op(A1_ENG[2], 2, 0, 3)   # needs phi6..11
                q_op(1, a1_1, a2_1)
                q_op(2, a1_2, a2_2)

            def emit_body():
                # ---------------- layer 0 ----------------
                BB = BC // 4   # 256; 4 batch-blocks on partition quarters
                rt4 = const.tile([128, BB], F32, name="rt4")
                for k in range(4):
                    nc.sync.dma_start(rt4[32 * k:32 * k + IN0, :],
                                      rt_d.ap()[:, k * BB:(k + 1) * BB])

                f0 = [
                    f0p.tile([128, BC], F32R, name="f0a"),
                    f0p.tile([128, BC], F32R, name="f0b"),
                    f0p.tile([64, BC], F32R, name="f0c"),
                ]
                # pad rows must be finite; ACT copy is an f32r-rounding
                # producer (BIR verifier requires one for f32r matmul inputs)
                zt = wconst.tile([128, BC], F32, name="zt")
                nc.vector.memset(zt[:], 0.0)
                for t in f0:
                    p = t.shape[0]
                    nc.scalar.copy(t[:], zt[0:p, :])

                sil0 = silp.tile([128, BB], F32, name="sil0")
                nc.scalar.activation(sil0[:], rt4[:], AFT.Silu)
                for k in range(4):
                    nc.scalar.copy(f0[0][0:IN0, k * BB:(k + 1) * BB],
                                   sil0[32 * k:32 * k + IN0, :])

                # features -> per-group q staging tiles, then ACT copies
                q0_tiles = {}

                def qt_out0(g, gw):
                    t = qfp.tile([128, 3 * 512], F32, name="qf")
                    q0_tiles[g] = t
                    return t[:, 0:gw]

                emit_features(rt4[:], BB, qt_out0, cvt=None)
                for g in range(3):
                    q0 = q0_tiles[g]
                    for jj in range(3):
                        j = 3 * g + jj
                        row = 32 * (1 + j)
                        ti, off = row // 128, row % 128
                        for k in range(4):
                            nc.scalar.copy(
                                f0[ti][off:off + IN0, k * BB:(k + 1) * BB],
                                q0[32 * k:32 * k + IN0,
                                   jj * BB:(jj + 1) * BB])

                w0 = [
                    wconst.tile([128, HID], F32R, name="w0a"),
                    wconst.tile([128, HID], F32R, name="w0b"),
                    wconst.tile([64, HID], F32R, name="w0c"),
                ]
                # split w0 DMA into 32-row chunks to spread across DMA engines
                for t, base in ((w0[0], 0), (w0[1], 128)):
                    for k in range(4):
                        nc.sync.dma_start(
                            t[32 * k:32 * (k + 1), :],
                            w0_d.ap()[base + 32 * k:base + 32 * (k + 1), :])
                for k in range(2):
                    nc.sync.dma_start(
                        w0[2][32 * k:32 * (k + 1), :],
                        w0_d.ap()[256 + 32 * k:256 + 32 * (k + 1), :])

                hT = [hp.tile([128, BC], BF16, name="hT") for _ in range(8)]
                for bh in range(2):
                    bsl = slice(bh * 512, (bh + 1) * 512)
                    for o in range(8):
                        ps = psp.tile([128, 512], F32, name="ps")
                        for t in range(3):
                            nc.tensor.matmul(
                                ps[:], w0[t][:, o * 128:(o + 1) * 128],
                                f0[t][:, bsl], start=(t == 0), stop=(t == 2))
                        nc.scalar.copy(hT[o][:, bsl], ps[:])

                # ---------------- layer 1 ----------------
                for ch in range(2):
                    bsl = slice(ch * 512, (ch + 1) * 512)
                    pss = [psp.tile([128, 512], F32, name="ps")
                           for _ in range(8)]
                    for it in range(8):
                        x = hT[it][:, bsl]
                        sil = silp.tile([128, 512], BF16, name="sil")
                        nc.scalar.activation(sil[:], x, AFT.Silu)
                        qt = qtp.tile([128, NJ * 512], BF16, name="qt")

                        def qt_out(g, gw):
                            return qt[:, 3 * g * 512:3 * g * 512 + gw]

                        emit_features(x, 512, qt_out, cvt='act')
                        for s in range(NSEC):
                            c = it * NSEC + s
                            w = w1p.tile([128, OUT], BF16, name="w1")
                            nc.sync.dma_start(w[:], w1_d.ap()[c, :, :])
                            F = sil[:] if s == 0 else qt[:, (s - 1) * 512:s * 512]
                            for o in range(8):
                                nc.tensor.matmul(
                                    pss[o][:], w[:, o * 128:(o + 1) * 128], F,
                                    start=(c == 0), stop=(c == C1_TILES - 1))
                    for o in range(8):
                        st = osp.tile([128, 512], F32, name="ost")
                        nc.scalar.copy(st[:], pss[o][:])
                        nc.sync.dma_start(
                            out_d.ap()[o * 128:(o + 1) * 128, bsl], st[:])

            loop_cm = (tc.For_i(0, iters // unroll, 1, hint_engines=(
                mybir.EngineType.PE, mybir.EngineType.DVE,
                mybir.EngineType.Activation, mybir.EngineType.Pool,
                mybir.EngineType.SP)) if iters > 1 else None)
            if loop_cm is not None:
                loop_cm.__enter__()
            for _ in range(unroll):
                emit_body()
            if loop_cm is not None:
                loop_cm.__exit__(None, None, None)

    nc.compile()
    _PROGRAMS[iters] = nc
    return nc


# ---------------------------------------------------------------- host driver
def prepare_inputs(x, bw0, sw0, sc0, bw1, sw1, sc1):
    """Host-side prep: roots + folded weights + per-core in_maps."""
    roots = _poly_roots_host(np.asarray(x, np.float32))          # [B, 20]
    rootsT = np.ascontiguousarray(roots.T)                        # [20, B]

    W0 = _fold_weights(bw0, sw0, sc0)                             # [1024, 20, 10]
    W1 = _fold_weights(bw1, sw1, sc1)                             # [1024, 1024, 10]

    # layer-0 DRAM layout [320, 1024] f32r: row 32*s + i -> W0[o, i, s]
    w0t = np.zeros((C0_ROWS, HID), np.float64)
    for s in range(NSEC):
        w0t[32 * s:32 * s + IN0, :] = W0[:, :, s].T
    w0t = _round_f32r(w0t.astype(np.float32))

    # layer-1 DRAM layout [80, 128, 1024] bf16: (c = it*10 + s, p, o)
    w1t = np.empty((C1_TILES, 128, OUT), _BF16)
    for it in range(8):
        blk = W1[:, it * 128:(it + 1) * 128, :]                   # [O, 128, 10]
        for s in range(NSEC):
            w1t[it * NSEC + s] = blk[:, :, s].T.astype(_BF16)

    in_maps = []
    for c in range(CORES):
        in_maps.append({
            "rt": np.ascontiguousarray(rootsT[:, c * BC:(c + 1) * BC]),
            "w0t": w0t,
            "w1t": w1t,
        })
    return in_maps


def assemble_output(results):
    """Per-core [OUT, BC] (o, b) outputs -> full [B, OUT]."""
    return np.ascontiguousarray(
        np.concatenate([np.asarray(r["out"]).T for r in results], axis=0)
    ).astype(np.float32)


def kernel(x, bw0, sw0, sc0, bw1, sw1, sc1):
    from concourse.bass_utils import run_bass_kernel_spmd
    args = [np.asarray(a, np.float32) for a in (x, bw0, sw0, sc0, bw1, sw1, sc1)]
    nc = build_program()
    in_maps = prepare_inputs(*args)
    res = run_bass_kernel_spmd(nc, in_maps, list(range(CORES)))
    return assemble_output(res.results)
